# revision 14
# baseline (speedup 1.0000x reference)
"""Piecewise-linear activation (uniform 16-point grid) on 8 trn2 NeuronCores.

Math: the reference is (except at exact grid hits) the continuous PWL function
    f(x) = A*x + B + sum_k c_k * relu(x - xs_k),   k = 0..15
with
    m_j   = (ys[j+1]-ys[j])/(xs[j+1]-xs[j])      (15 interior slopes)
    c_0   = m_0 - slopes[0]
    c_k   = m_k - m_{k-1}                         (k=1..14)
    c_15  = slopes[1] - m_14
    A     = slopes[0],  B = ys[0] - slopes[0]*xs[0]
At an exact interior grid hit x == xs[j] (j=1..15) the reference's
argmin/argmax tie-breaking yields the *two-segment-wide* interpolation value
(discontinuous from f); those are patched via compare/select fixup slots.

Engine split per 128x2048 supertile (all scales folded so no extra passes):
  ACT    : affine pass (Identity) + 6 scaled relu terms relu(c_k*(x-xs_k))
           (negative c_k flips the hinge; its affine part is folded into A,B)
  DVE    : 8 unscaled relu terms (tensor_scalar (x - xs_k) max 0, 2x fp32 mode)
  GPSIMD : 2 unscaled relu terms (scalar_tensor_tensor with a zeros tile)
  PE     : accumulates everything into PSUM via identity matmuls;
           unscaled terms use c_k*I weights (scale rides on the matmul)
  ACT    : PSUM -> SBUF copy (DMA cannot read PSUM)
  DVE    : N_SLOTS exact-hit fixups (is_equal mask + copy_predicated)
"""

import numpy as np
from contextlib import ExitStack

import concourse.bass as bass
import concourse.bacc as bacc
import concourse.tile as tile
from concourse import mybir
from concourse.bass_utils import run_bass_kernel_spmd

F32 = mybir.dt.float32
AF = mybir.ActivationFunctionType
OP = mybir.AluOpType

N_CORES = 8
P = 128
FREE = 8192              # elements per partition per core (1024*1024/128)
ST = 2048                # supertile free size
CHUNK = 512              # one PSUM bank worth of fp32

ACT_KS = [0, 1, 2, 3, 4, 5]
DVE_KS = [6, 7, 8, 9, 10, 11, 12, 13, 14, 15]
GP_KS = []
N_SLOTS = 2              # device-side exact-hit fixup slots

# consts column layout (each column is one value replicated over 128 partitions)
#  0: A, 1: B,
#  2..2+2*6:   (scale, bias) per ACT term
#  14..14+8:   xs_k per DVE term
#  22..22+2:   xs_k per GP term
#  24..24+2:   fixup g values (NaN when unused)
#  26..26+2:   fixup v values
NCOL = 32
COL_A, COL_B = 0, 1
COL_ACT = 2
COL_DVE = COL_ACT + 2 * len(ACT_KS)
COL_GP = COL_DVE + len(DVE_KS)
COL_G = COL_GP + len(GP_KS)
COL_V = COL_G + N_SLOTS

NW = 2 + len(DVE_KS) + len(GP_KS)   # weight mats: I, -I, then c_k*I per unscaled term

_CACHE = {}


def _build_program():
    nc = bacc.Bacc(None, target_bir_lowering=False, debug=False)
    xin = nc.declare_dram_parameter("xin", [P, FREE], F32, isOutput=False)
    consts = nc.declare_dram_parameter("consts", [P, NCOL], F32, isOutput=False)
    wmats = nc.declare_dram_parameter("wmats", [P, NW * P], F32, isOutput=False)
    xout = nc.declare_dram_parameter("xout", [P, FREE], F32, isOutput=True)

    with tile.TileContext(nc) as tc, ExitStack() as ctx:
        const_pool = ctx.enter_context(tc.tile_pool(name="const", bufs=1))
        x_pool = ctx.enter_context(tc.tile_pool(name="x", bufs=2))
        term_pool = ctx.enter_context(tc.tile_pool(name="term", bufs=6))
        out_pool = ctx.enter_context(tc.tile_pool(name="out", bufs=2))
        mask_pool = ctx.enter_context(tc.tile_pool(name="mask", bufs=2))
        psum_pool = ctx.enter_context(
            tc.tile_pool(name="psum", bufs=2, space=bass.MemorySpace.PSUM)
        )

        cs = const_pool.tile([P, NCOL], F32)
        nc.sync.dma_start(cs[:], consts[:])
        ws = const_pool.tile([P, NW * P], F32)
        nc.sync.dma_start(ws[:], wmats[:])

        zeros = const_pool.tile([P, ST], F32)
        nc.vector.memset(zeros[:], 0.0)
        vtiles = []
        for s in range(N_SLOTS):
            vt = const_pool.tile([P, ST], F32, tag=f"vtile{s}")
            nc.vector.tensor_scalar(
                vt[:], zeros[:], cs[:, COL_V + s : COL_V + s + 1], None, OP.add
            )
            vtiles.append(vt)

        col = lambda i: cs[:, i : i + 1]
        wmat = lambda i: ws[:, i * P : (i + 1) * P]

        for st in range(FREE // ST):
            sl = slice(st * ST, (st + 1) * ST)
            xt = x_pool.tile([P, ST], F32)
            nc.sync.dma_start(xt[:], xin[:, sl])

            # (tile, weight-mat index): 0 -> +I, 1 -> -I, 2+i -> c_k*I.
            # ACT term i uses scale=|c_k| so its sign rides on the +-I pick;
            # sign indices come from the host via _CACHE["act_signs"].
            act_signs = _CACHE["act_signs"]
            rhss = []
            aff = term_pool.tile([P, ST], F32, tag="term")
            nc.scalar.activation(
                aff[:], xt[:], AF.Identity, bias=col(COL_B), scale=col(COL_A)
            )
            rhss.append((aff, 0))
            for i, _ in enumerate(ACT_KS):
                t = term_pool.tile([P, ST], F32, tag="term")
                nc.scalar.activation(
                    t[:], xt[:], AF.Relu,
                    bias=col(COL_ACT + 2 * i + 1), scale=col(COL_ACT + 2 * i),
                )
                rhss.append((t, act_signs[i]))
            for i, _ in enumerate(DVE_KS):
                t = term_pool.tile([P, ST], F32, tag="term")
                nc.vector.tensor_scalar(
                    t[:], xt[:], col(COL_DVE + i), 0.0, OP.subtract, OP.max
                )
                rhss.append((t, 2 + i))
            for i, _ in enumerate(GP_KS):
                t = term_pool.tile([P, ST], F32, tag="term")
                nc.gpsimd.scalar_tensor_tensor(
                    t[:], xt[:], col(COL_GP + i), zeros[:], OP.subtract, OP.max
                )
                rhss.append((t, 2 + len(DVE_KS) + i))
            # group matmuls by weight mat so walrus can batch LDWEIGHTS
            rhss.sort(key=lambda p: p[1])

            ps = psum_pool.tile([P, ST], F32)
            n_terms = len(rhss)
            for ti, (t, wi) in enumerate(rhss):
                for c in range(ST // CHUNK):
                    cc = slice(c * CHUNK, (c + 1) * CHUNK)
                    nc.tensor.matmul(
                        ps[:, cc], wmat(wi), t[:, cc],
                        start=(ti == 0), stop=(ti == n_terms - 1),
                    )

            ot = out_pool.tile([P, ST], F32)
            nc.scalar.copy(ot[:], ps[:])

            for s in range(N_SLOTS):
                mk = mask_pool.tile([P, ST], mybir.dt.uint8, tag="mask")
                nc.vector.tensor_scalar(
                    mk[:], xt[:], col(COL_G + s), None, OP.is_equal
                )
                nc.vector.copy_predicated(ot[:], mk[:], vtiles[s][:])

            nc.sync.dma_start(xout[:, sl], ot[:])

    nc.compile()
    return nc


def _host_consts(x, xs, slopes, ys):
    xs64 = xs.astype(np.float64)
    ys64 = ys.astype(np.float64)
    s0, s1 = float(slopes[0]), float(slopes[1])
    m = np.diff(ys64) / np.diff(xs64)
    c = np.empty(16, np.float64)
    c[0] = m[0] - s0
    c[1:15] = np.diff(m)
    c[15] = s1 - m[14]
    A = s0
    B = ys64[0] - s0 * xs64[0]

    consts = np.zeros(NCOL, np.float64)
    for i, k in enumerate(ACT_KS):
        consts[COL_ACT + 2 * i] = abs(c[k])
        consts[COL_ACT + 2 * i + 1] = -abs(c[k]) * xs64[k]
    for i, k in enumerate(DVE_KS):
        consts[COL_DVE + i] = xs64[k]
    for i, k in enumerate(GP_KS):
        consts[COL_GP + i] = xs64[k]
    consts[COL_A] = A
    consts[COL_B] = B

    act_signs = tuple(0 if c[k] >= 0 else 1 for k in ACT_KS)

    # exact-hit fixups: reference's tie-breaking at x == xs[j], j=1..15
    consts[COL_G : COL_G + N_SLOTS] = np.nan
    host_patches = []
    slot = 0
    xs32, ys32 = xs.astype(np.float32), ys.astype(np.float32)
    for j in range(1, 16):
        g = xs32[j]
        if not np.any(x == g):
            continue
        l, r = (j - 1, j + 1) if j < 15 else (14, 0)
        denom = np.float32(xs32[r] - xs32[l])
        v = np.float32(
            ys32[l] + (g - xs32[l]) * np.float32(ys32[r] - ys32[l]) / denom
        )
        if slot < N_SLOTS:
            consts[COL_G + slot] = np.float64(g)
            consts[COL_V + slot] = np.float64(v)
            slot += 1
        else:
            host_patches.append((g, v))

    wm = np.zeros((NW, P, P), np.float64)
    wm[0] = np.eye(P)
    wm[1] = -np.eye(P)
    for i, k in enumerate(DVE_KS + GP_KS):
        wm[2 + i] = c[k] * np.eye(P)

    consts_np = np.broadcast_to(
        consts.astype(np.float32), (P, NCOL)
    ).copy()
    # wmats DRAM layout [P, NW*P]: partition p holds row p of every weight mat
    wmats_np = np.ascontiguousarray(
        wm.astype(np.float32).transpose(1, 0, 2).reshape(P, NW * P)
    )
    return consts_np, wmats_np, host_patches, act_signs


def kernel(x, xs, slopes, ys):
    x = np.ascontiguousarray(x, np.float32)
    consts_np, wmats_np, host_patches, act_signs = _host_consts(x, xs, slopes, ys)
    if _CACHE.get("act_signs") != act_signs:
        _CACHE["act_signs"] = act_signs
        _CACHE["nc"] = _build_program()
    nc = _CACHE["nc"]
    shards = x.reshape(N_CORES, P, FREE)
    in_maps = [
        {"xin": shards[i], "consts": consts_np, "wmats": wmats_np}
        for i in range(N_CORES)
    ]
    import os
    res = run_bass_kernel_spmd(
        nc, in_maps, list(range(N_CORES)),
        trace=bool(int(os.environ.get("KERNEL_TRACE", "0"))),
    )
    _CACHE["last_results"] = res
    out = np.stack([res.results[i]["xout"] for i in range(N_CORES)])
    out = out.reshape(x.shape)
    for g, v in host_patches:  # only if >N_SLOTS distinct exact-hit values
        out[x == g] = v
    return out


# revision 15
# speedup vs baseline: 1.8527x; 1.8527x over previous
"""Piecewise-linear activation (uniform 16-point grid) on 8 trn2 NeuronCores.

Math: the reference is (except at exact grid hits) the continuous PWL function
    f(x) = A*x + B + sum_k c_k * relu(x - xs_k),   k = 0..15
with
    m_j   = (ys[j+1]-ys[j])/(xs[j+1]-xs[j])      (15 interior slopes)
    c_0   = m_0 - slopes[0]
    c_k   = m_k - m_{k-1}                         (k=1..14)
    c_15  = slopes[1] - m_14
    A     = slopes[0],  B = ys[0] - slopes[0]*xs[0]
At an exact interior grid hit x == xs[j] (j=1..15) the reference's
argmin/argmax tie-breaking yields the two-segment-wide interpolation value
(discontinuous from f); those are patched via compare/select fixup slots.

Engine split per 128x2048 supertile (measured per-instr costs drove this):
  ACT  : affine pass (Identity) + most relu produces (unscaled, bias=-xs_k)
  DVE  : a few relu produces (tensor_scalar 2x mode), plus the scaled
         accumulation chain via affine_then_add (acc += c_k * r_k),
         PSUM merge, and the exact-hit fixups (is_equal + copy_predicated)
  PE   : remaining terms accumulate into PSUM via c_k*I fp32 matmuls
  (GPSIMD tensor ops are rejected by walrus codegen / crash the NC - unused)
"""

import numpy as np
from contextlib import ExitStack

import concourse.bass as bass
import concourse.bacc as bacc
import concourse.tile as tile
from concourse import mybir
from concourse.bass_utils import run_bass_kernel_spmd

F32 = mybir.dt.float32
AF = mybir.ActivationFunctionType
OP = mybir.AluOpType

N_CORES = 8
P = 128
FREE = 8192              # elements per partition per core (1024*1024/128)
ST = 2048                # supertile free size
CHUNK = 512              # one PSUM bank worth of fp32

# term k (0..15) -> producer engine and combiner engine.
# produces: 14 on ACT, 2 on DVE; combines: 9 on DVE (ATA), 7 on PE.
DVE_PROD_KS = [14, 15]
ATA_KS = [0, 1, 2, 3, 4, 5, 6, 7, 14]       # combined on DVE acc chain
PE_KS = [8, 9, 10, 11, 12, 13, 15]          # combined on PSUM
N_SLOTS = 2              # device-side exact-hit fixup slots

# consts column layout (each column replicated over 128 partitions):
#  0: A, 1: B, 2..18: xs_k bias (-xs_k) per term k, 18..34: c_k per term k,
#  34..36: fixup g, 36..38: fixup v
NCOL = 40
COL_A, COL_B = 0, 1
COL_BIAS = 2
COL_C = 18
COL_G = 34
COL_V = COL_G + N_SLOTS

NW = len(PE_KS)          # weight mats: c_k*I per PE-combined term

_CACHE = {}


def _build_program():
    nc = bacc.Bacc(None, target_bir_lowering=False, debug=False)
    xin = nc.declare_dram_parameter("xin", [P, FREE], F32, isOutput=False)
    consts = nc.declare_dram_parameter("consts", [P, NCOL], F32, isOutput=False)
    wmats = nc.declare_dram_parameter("wmats", [P, NW * P], F32, isOutput=False)
    xout = nc.declare_dram_parameter("xout", [P, FREE], F32, isOutput=True)

    with tile.TileContext(nc) as tc, ExitStack() as ctx:
        const_pool = ctx.enter_context(tc.tile_pool(name="const", bufs=1))
        x_pool = ctx.enter_context(tc.tile_pool(name="x", bufs=3))
        term_pool = ctx.enter_context(tc.tile_pool(name="term", bufs=6))
        acc_pool = ctx.enter_context(tc.tile_pool(name="acc", bufs=3))
        out_pool = ctx.enter_context(tc.tile_pool(name="out", bufs=3))
        mask_pool = ctx.enter_context(tc.tile_pool(name="mask", bufs=3))
        psum_pool = ctx.enter_context(
            tc.tile_pool(name="psum", bufs=2, space=bass.MemorySpace.PSUM)
        )

        cs = const_pool.tile([P, NCOL], F32)
        nc.sync.dma_start(cs[:], consts[:])
        ws = const_pool.tile([P, NW * P], F32)
        nc.sync.dma_start(ws[:], wmats[:])

        zeros = const_pool.tile([P, ST], F32)
        nc.vector.memset(zeros[:], 0.0)
        vtiles = []
        for s in range(N_SLOTS):
            vt = const_pool.tile([P, ST], F32, tag=f"vtile{s}")
            nc.vector.tensor_scalar(
                vt[:], zeros[:], cs[:, COL_V + s : COL_V + s + 1], None, OP.add
            )
            vtiles.append(vt)

        col = lambda i: cs[:, i : i + 1]
        wmat = lambda i: ws[:, i * P : (i + 1) * P]

        for st in range(FREE // ST):
            sl = slice(st * ST, (st + 1) * ST)
            xt = x_pool.tile([P, ST], F32)
            nc.sync.dma_start(xt[:], xin[:, sl])

            # unscaled relu produces: r_k = relu(x - xs_k)
            terms = {}
            for k in range(16):
                t = term_pool.tile([P, ST], F32, tag="term")
                if k in DVE_PROD_KS:
                    nc.vector.tensor_scalar(
                        t[:], xt[:], col(COL_BIAS + k), 0.0, OP.add, OP.max
                    )
                else:
                    nc.scalar.activation(
                        t[:], xt[:], AF.Relu, bias=col(COL_BIAS + k), scale=1.0
                    )
                terms[k] = t

            # DVE accumulation chain, seeded by the affine base from ACT
            acc = acc_pool.tile([P, ST], F32)
            nc.scalar.activation(
                acc[:], xt[:], AF.Identity, bias=col(COL_B), scale=col(COL_A)
            )
            for k in ATA_KS:
                nc.vector.affine_then_add(
                    acc[:], terms[k][:], acc[:], col(COL_C + k), 0.0
                )

            # PE accumulation into PSUM
            ps = psum_pool.tile([P, ST], F32)
            for ti, k in enumerate(PE_KS):
                for c in range(ST // CHUNK):
                    cc = slice(c * CHUNK, (c + 1) * CHUNK)
                    nc.tensor.matmul(
                        ps[:, cc], wmat(ti), terms[k][:, cc],
                        start=(ti == 0), stop=(ti == len(PE_KS) - 1),
                    )

            # merge psum into acc -> out
            ot = out_pool.tile([P, ST], F32)
            nc.vector.affine_then_add(ot[:], ps[:], acc[:], 1.0, 0.0)

            # exact-hit fixups
            for s in range(N_SLOTS):
                mk = mask_pool.tile([P, ST], mybir.dt.uint8, tag="mask")
                nc.vector.tensor_scalar(
                    mk[:], xt[:], col(COL_G + s), None, OP.is_equal
                )
                nc.vector.copy_predicated(ot[:], mk[:], vtiles[s][:])

            nc.sync.dma_start(xout[:, sl], ot[:])

    nc.compile()
    return nc


def _host_consts(x, xs, slopes, ys):
    xs64 = xs.astype(np.float64)
    ys64 = ys.astype(np.float64)
    s0, s1 = float(slopes[0]), float(slopes[1])
    m = np.diff(ys64) / np.diff(xs64)
    c = np.empty(16, np.float64)
    c[0] = m[0] - s0
    c[1:15] = np.diff(m)
    c[15] = s1 - m[14]

    consts = np.zeros(NCOL, np.float64)
    consts[COL_A] = s0
    consts[COL_B] = ys64[0] - s0 * xs64[0]
    for k in range(16):
        consts[COL_BIAS + k] = -xs64[k]
        consts[COL_C + k] = c[k]

    # exact-hit fixups: reference's tie-breaking at x == xs[j], j=1..15
    consts[COL_G : COL_G + N_SLOTS] = np.nan
    host_patches = []
    slot = 0
    xs32, ys32 = xs.astype(np.float32), ys.astype(np.float32)
    for j in range(1, 16):
        g = xs32[j]
        if not np.any(x == g):
            continue
        l, r = (j - 1, j + 1) if j < 15 else (14, 0)
        denom = np.float32(xs32[r] - xs32[l])
        v = np.float32(
            ys32[l] + (g - xs32[l]) * np.float32(ys32[r] - ys32[l]) / denom
        )
        if slot < N_SLOTS:
            consts[COL_G + slot] = np.float64(g)
            consts[COL_V + slot] = np.float64(v)
            slot += 1
        else:
            host_patches.append((g, v))

    wm = np.zeros((NW, P, P), np.float64)
    for i, k in enumerate(PE_KS):
        wm[i] = c[k] * np.eye(P)

    consts_np = np.broadcast_to(consts.astype(np.float32), (P, NCOL)).copy()
    # wmats DRAM layout [P, NW*P]: partition p holds row p of every weight mat
    wmats_np = np.ascontiguousarray(
        wm.astype(np.float32).transpose(1, 0, 2).reshape(P, NW * P)
    )
    return consts_np, wmats_np, host_patches


def kernel(x, xs, slopes, ys):
    x = np.ascontiguousarray(x, np.float32)
    consts_np, wmats_np, host_patches = _host_consts(x, xs, slopes, ys)
    if "nc" not in _CACHE:
        _CACHE["nc"] = _build_program()
    nc = _CACHE["nc"]

    shards = x.reshape(N_CORES, P, FREE)
    in_maps = [
        {"xin": shards[i], "consts": consts_np, "wmats": wmats_np}
        for i in range(N_CORES)
    ]
    import os
    res = run_bass_kernel_spmd(
        nc, in_maps, list(range(N_CORES)),
        trace=bool(int(os.environ.get("KERNEL_TRACE", "0"))),
    )
    _CACHE["last_results"] = res
    out = np.stack([res.results[i]["xout"] for i in range(N_CORES)])
    out = out.reshape(x.shape)
    for g, v in host_patches:  # only if >N_SLOTS distinct exact-hit values
        out[x == g] = v
    return out


# revision 16
# speedup vs baseline: 2.0493x; 1.1061x over previous
"""Piecewise-linear activation (uniform 16-point grid) on 8 trn2 NeuronCores.

Math: the reference is (except at exact grid hits) the continuous PWL function
    f(x) = A*x + B + sum_k c_k * relu(x - xs_k),   k = 0..15
with
    m_j   = (ys[j+1]-ys[j])/(xs[j+1]-xs[j])      (15 interior slopes)
    c_0   = m_0 - slopes[0]
    c_k   = m_k - m_{k-1}                         (k=1..14)
    c_15  = slopes[1] - m_14
    A     = slopes[0],  B = ys[0] - slopes[0]*xs[0]
At an exact interior grid hit x == xs[j] (j=1..15) the reference's
argmin/argmax tie-breaking yields the two-segment-wide interpolation value
(discontinuous from f); those are patched with select fixup slots.

Engine split per 128x2048 supertile (driven by measured per-instr costs and
the fact that the PE array is HAM power-throttled and fp32 matmul is 2-pass):
  ACT : affine chain seed (Identity) + unscaled relu produces for PE terms
  DVE : RELU_MAC custom ops (acc = relu(x + b)*c + acc) for most terms -
        produce+scale+accumulate fused into one 1x pass per term - plus the
        PSUM merge (affine_then_add) and EQ_SELECT fixup ops
  PE  : remaining terms accumulate into PSUM via c_k*I fp32 matmuls
  (GPSIMD tensor ops are rejected by walrus codegen / crash the NC - unused)
"""

import numpy as np
from contextlib import ExitStack

import concourse.bass as bass
import concourse.bacc as bacc
import concourse.tile as tile
from concourse import mybir
from concourse.bass_utils import run_bass_kernel_spmd

F32 = mybir.dt.float32
AF = mybir.ActivationFunctionType
OP = mybir.AluOpType

N_CORES = 8
P = 128
FREE = 8192              # elements per partition per core (1024*1024/128)
ST = 2048                # supertile free size
CHUNK = 512              # one PSUM bank worth of fp32

# term k (0..15): DVE_KS fused on the DVE RELU_MAC chain, PE_KS via PSUM
DVE_KS = [0, 1, 2, 3, 4, 5, 6, 7, 8, 9]
PE_KS = [10, 11, 12, 13, 14, 15]
N_SLOTS = 2              # exact-hit fixup slots
UNUSED_G = 1.0e30        # sentinel no input value ever equals

# consts column layout (each column replicated over 128 partitions):
NCOL = 40
COL_A, COL_B = 0, 1
COL_BIAS = 2             # 16 cols: -xs_k
COL_C = 18               # 16 cols: c_k
COL_G = 34               # N_SLOTS fixup compare values
COL_V = COL_G + N_SLOTS  # N_SLOTS fixup replacement values

NW = len(PE_KS)          # weight mats: c_k*I per PE-combined term

_CACHE = {}


def _register_custom_ops():
    if "ops" in _CACHE:
        return _CACHE["ops"]
    import concourse.dve_ops as dve_ops
    from concourse.dve_spec import Spec, Src0, Src1, C0, C1, relu, select, eq, lower
    from concourse.dve_spec import _has_src1
    from concourse.dve_uop import DveOpSpec

    def make(name, spec):
        row = dve_ops._CUSTOM_DVE_ROW_BASE + len(dve_ops.OPS)
        shas = {}
        for ver in ("v3", "v4"):
            s = DveOpSpec(name=name, opcode=row,
                          uops=lower(spec, ver=ver), rd1_en=_has_src1(spec))
            shas[ver] = s.sha(ver)
        op = dve_ops.DveOp(name, spec, subdim=False, uops_sha=shas)
        dve_ops.OPS.append(op)
        dve_ops._SUB_OPCODE_FOR_NAME[name] = row
        dve_ops.CUSTOM_DVE_SPECS[name] = spec
        return op

    relu_mac = make("RELU_MAC_PWA", Spec(
        body=relu(Src0 + C0) * C1 + Src1,
        reference=lambda in0, in1, s0, s1, imm2:
            np.maximum(in0.astype(np.float32) + s0, 0) * s1 + in1,
    ))
    eq_sel = make("EQ_SELECT_PWA", Spec(
        body=select(eq(Src0, C0), C1, Src1),
        reference=lambda in0, in1, s0, s1, imm2:
            np.where(in0 == s0, np.float32(s1), in1).astype(np.float32),
    ))
    _CACHE["ops"] = (relu_mac, eq_sel)
    return _CACHE["ops"]


def _build_program():
    relu_mac, eq_sel = _register_custom_ops()
    nc = bacc.Bacc(None, target_bir_lowering=False, debug=False)
    xin = nc.declare_dram_parameter("xin", [P, FREE], F32, isOutput=False)
    consts = nc.declare_dram_parameter("consts", [P, NCOL], F32, isOutput=False)
    wmats = nc.declare_dram_parameter("wmats", [P, NW * P], F32, isOutput=False)
    xout = nc.declare_dram_parameter("xout", [P, FREE], F32, isOutput=True)

    with tile.TileContext(nc) as tc, ExitStack() as ctx:
        const_pool = ctx.enter_context(tc.tile_pool(name="const", bufs=1))
        x_pool = ctx.enter_context(tc.tile_pool(name="x", bufs=3))
        term_pool = ctx.enter_context(tc.tile_pool(name="term", bufs=6))
        acc_pool = ctx.enter_context(tc.tile_pool(name="acc", bufs=3))
        out_pool = ctx.enter_context(tc.tile_pool(name="out", bufs=3))
        psum_pool = ctx.enter_context(
            tc.tile_pool(name="psum", bufs=2, space=bass.MemorySpace.PSUM)
        )

        cs = const_pool.tile([P, NCOL], F32)
        nc.sync.dma_start(cs[:], consts[:])
        ws = const_pool.tile([P, NW * P], F32)
        nc.sync.dma_start(ws[:], wmats[:])

        col = lambda i: cs[:, i : i + 1]
        wmat = lambda i: ws[:, i * P : (i + 1) * P]

        for st in range(FREE // ST):
            sl = slice(st * ST, (st + 1) * ST)
            xt = x_pool.tile([P, ST], F32)
            nc.sync.dma_start(xt[:], xin[:, sl])

            # ACT: unscaled relu produces for the PE-combined terms
            terms = {}
            for k in PE_KS:
                t = term_pool.tile([P, ST], F32, tag="term")
                nc.scalar.activation(
                    t[:], xt[:], AF.Relu, bias=col(COL_BIAS + k), scale=1.0
                )
                terms[k] = t

            # PE accumulation into PSUM
            ps = psum_pool.tile([P, ST], F32)
            for ti, k in enumerate(PE_KS):
                for c in range(ST // CHUNK):
                    cc = slice(c * CHUNK, (c + 1) * CHUNK)
                    nc.tensor.matmul(
                        ps[:, cc], wmat(ti), terms[k][:, cc],
                        start=(ti == 0), stop=(ti == len(PE_KS) - 1),
                    )

            # DVE fused chain, seeded by the affine base from ACT
            acc = acc_pool.tile([P, ST], F32)
            nc.scalar.activation(
                acc[:], xt[:], AF.Identity, bias=col(COL_B), scale=col(COL_A)
            )
            for k in DVE_KS:
                nc.vector._custom_dve(
                    relu_mac, out=acc[:], in0=xt[:], in1=acc[:],
                    s0=col(COL_BIAS + k), s1=col(COL_C + k),
                )

            # merge psum into acc -> out, then exact-hit fixups
            ot = out_pool.tile([P, ST], F32)
            nc.vector.affine_then_add(ot[:], ps[:], acc[:], 1.0, 0.0)
            for s in range(N_SLOTS):
                nc.vector._custom_dve(
                    eq_sel, out=ot[:], in0=xt[:], in1=ot[:],
                    s0=col(COL_G + s), s1=col(COL_V + s),
                )

            nc.sync.dma_start(xout[:, sl], ot[:])

    nc.compile()
    return nc


def _host_consts(x, xs, slopes, ys):
    xs64 = xs.astype(np.float64)
    ys64 = ys.astype(np.float64)
    s0, s1 = float(slopes[0]), float(slopes[1])
    m = np.diff(ys64) / np.diff(xs64)
    c = np.empty(16, np.float64)
    c[0] = m[0] - s0
    c[1:15] = np.diff(m)
    c[15] = s1 - m[14]

    consts = np.zeros(NCOL, np.float64)
    consts[COL_A] = s0
    consts[COL_B] = ys64[0] - s0 * xs64[0]
    for k in range(16):
        consts[COL_BIAS + k] = -xs64[k]
        consts[COL_C + k] = c[k]

    # exact-hit fixups: reference's tie-breaking at x == xs[j], j=1..15
    consts[COL_G : COL_G + N_SLOTS] = UNUSED_G
    host_patches = []
    slot = 0
    xs32, ys32 = xs.astype(np.float32), ys.astype(np.float32)
    for j in range(1, 16):
        g = xs32[j]
        if not np.any(x == g):
            continue
        l, r = (j - 1, j + 1) if j < 15 else (14, 0)
        denom = np.float32(xs32[r] - xs32[l])
        v = np.float32(
            ys32[l] + (g - xs32[l]) * np.float32(ys32[r] - ys32[l]) / denom
        )
        if slot < N_SLOTS:
            consts[COL_G + slot] = np.float64(g)
            consts[COL_V + slot] = np.float64(v)
            slot += 1
        else:
            host_patches.append((g, v))

    wm = np.zeros((NW, P, P), np.float64)
    for i, k in enumerate(PE_KS):
        wm[i] = c[k] * np.eye(P)

    consts_np = np.broadcast_to(consts.astype(np.float32), (P, NCOL)).copy()
    # wmats DRAM layout [P, NW*P]: partition p holds row p of every weight mat
    wmats_np = np.ascontiguousarray(
        wm.astype(np.float32).transpose(1, 0, 2).reshape(P, NW * P)
    )
    return consts_np, wmats_np, host_patches


def kernel(x, xs, slopes, ys):
    x = np.ascontiguousarray(x, np.float32)
    consts_np, wmats_np, host_patches = _host_consts(x, xs, slopes, ys)
    if "nc" not in _CACHE:
        _CACHE["nc"] = _build_program()
    nc = _CACHE["nc"]

    shards = x.reshape(N_CORES, P, FREE)
    in_maps = [
        {"xin": shards[i], "consts": consts_np, "wmats": wmats_np}
        for i in range(N_CORES)
    ]
    import os
    res = run_bass_kernel_spmd(
        nc, in_maps, list(range(N_CORES)),
        trace=bool(int(os.environ.get("KERNEL_TRACE", "0"))),
    )
    _CACHE["last_results"] = res
    out = np.stack([res.results[i]["xout"] for i in range(N_CORES)])
    out = out.reshape(x.shape)
    for g, v in host_patches:  # only if >N_SLOTS distinct exact-hit values
        out[x == g] = v
    return out
